# revision 7
# baseline (speedup 1.0000x reference)
"""Trainium2 Bass kernel for the CircularSplineLayer problem.

Strategy: model-parallel over S_OUT (each of the 8 cores owns a 512-column
slice of the spline outputs and the matching 1/8 slice of w2/b2), with the
first layer (global normalise + Linear+tanh over x_passive) computed
redundantly on every core so no cross-core collectives are needed.

Per-core device program:
  phase A: global mean/std of x_passive (PE ones-matmul sums + ACT Square
           accumulation), then h = tanh(((x-m)/s) @ w1 + b1) via the
           linearity trick h = tanh((x @ w1)/s + (b1 - (m/s)*colsum(w1))),
           keeping h resident in SBUF in [HID, B] (transposed) layout.
  phase B: second matmul out = h @ w2_slice with w2 columns permuted
           j-major on the host, so each 128-partition output chunk holds a
           single spline-component plane [s x b].  The rational-quadratic
           spline evaluation then becomes pure elementwise plane algebra:
           exp/cumsum over the 8 w/h components, monotone-mask binning
           (cw_j < x), predicated-copy gathers, and the closed-form
           phi/grad math.  log-density partials reduce over s via a PE
           ones-matmul (partition-axis reduction).

Host: shards/permutes inputs, concatenates phi column slices, and sums the
8 per-core log-grad partials into log_density.
"""

import numpy as np

B, S_IN, S_OUT, K, HID = 1024, 4096, 4096, 8, 512
NCORES = 8
S_LOC = S_OUT // NCORES          # 512 spline outputs per core
NJ = 3 * K                       # 24 component planes per s
SP_TILES = S_LOC // 128          # 4 partition tiles of s per core
TWO_PI = float(2.0 * np.pi)
INV_2PI = float(1.0 / TWO_PI)
N_ELEMS = float(B * S_IN)        # normalisation count for mean/std

_CACHE = {}


def _build_program():
    import concourse.bass as bass
    import concourse.mybir as mybir
    from concourse import bacc
    from concourse.tile import TileContext
    from contextlib import ExitStack

    dt = mybir.dt
    f32 = dt.float32
    f32r = dt.float32r
    AF = mybir.ActivationFunctionType
    OP = mybir.AluOpType

    nc = bacc.Bacc("TRN2", target_bir_lowering=False, debug=False,
                   num_devices=NCORES)

    xT = nc.dram_tensor("xT", [S_IN, B], f32, kind="ExternalInput").ap()
    w1 = nc.dram_tensor("w1", [S_IN, HID], f32, kind="ExternalInput").ap()
    b1r = nc.dram_tensor("b1r", [HID // 128, 128], f32, kind="ExternalInput").ap()
    w2p = nc.dram_tensor("w2p", [HID, NJ * S_LOC], f32, kind="ExternalInput").ap()
    b2p = nc.dram_tensor("b2p", [NJ, S_LOC], f32, kind="ExternalInput").ap()
    xinT = nc.dram_tensor("xinT", [S_LOC, B], f32, kind="ExternalInput").ap()
    ph = nc.dram_tensor("ph", [1, 1], f32, kind="ExternalInput").ap()
    phiT = nc.dram_tensor("phiT", [S_LOC, B], f32, kind="ExternalOutput").ap()
    ldp = nc.dram_tensor("ldp", [1, B], f32, kind="ExternalOutput").ap()

    r = lambda ap: ap  # plain fp32 matmuls for now
    KC = S_IN // 128             # 32 contraction chunks for matmul 1
    MH = HID // 128              # 4 hid chunks
    BH = B // 512                # 2 b-halves

    with TileContext(nc) as tc, ExitStack() as ctx:
        const = ctx.enter_context(tc.tile_pool(name="const", bufs=1))
        hpool = ctx.enter_context(tc.tile_pool(name="hres", bufs=1))

        ones = const.tile([128, 1], f32, tag="ones")
        nc.vector.memset(ones, 1.0)
        b1sb = const.tile([128, MH], f32, tag="b1sb")
        for m in range(MH):
            nc.sync.dma_start(out=b1sb[:, m:m + 1], in_=b1r[m, :].rearrange("(p one) -> p one", one=1))
        b2sb = const.tile([128, NJ * SP_TILES], f32, tag="b2sb")
        for j in range(NJ):
            for sp in range(SP_TILES):
                nc.sync.dma_start(
                    out=b2sb[:, j * SP_TILES + sp: j * SP_TILES + sp + 1],
                    in_=b2p[j, sp * 128:(sp + 1) * 128].rearrange("(p one) -> p one", one=1))
        phsb = const.tile([128, 1], f32, tag="phsb")
        nc.sync.dma_start(out=phsb, in_=ph.broadcast_to((128, 1)))

        # h kept resident for all of phase B: [HID,B] as 4 tiles of [128,B]
        hres = [hpool.tile([128, B], f32, tag=f"h{m}", name=f"h{m}") for m in range(MH)]

        # scalar stats scratch
        stat = const.tile([128, 8], f32, tag="stat")  # cols: see below
        sqacc = const.tile([128, 2 * KC], f32, tag="sqacc")

        # ---------------- phase A: stats over x_passive ----------------
        with ExitStack() as actx:
            xpool = actx.enter_context(tc.tile_pool(name="xa", bufs=6))
            sqpool = actx.enter_context(tc.tile_pool(name="sq", bufs=3))
            pspool = actx.enter_context(tc.tile_pool(name="psA", bufs=1, space="PSUM"))

            s1ps = pspool.tile([1, 512], f32, tag="s1ps")
            i = 0
            for half in range(BH):
                for k in range(KC):
                    xt = xpool.tile([128, 512], f32, tag="xt")
                    nc.sync.dma_start(out=xt, in_=xT[k * 128:(k + 1) * 128,
                                                     half * 512:(half + 1) * 512])
                    sq = sqpool.tile([128, 512], f32, tag="sq")
                    nc.scalar.activation(out=sq, in_=xt, func=AF.Square,
                                         accum_out=sqacc[:, i:i + 1])
                    nc.tensor.matmul(s1ps, r(ones), r(xt),
                                     start=(i == 0), stop=(i == 2 * KC - 1))
                    i += 1
            # total sum -> stat[:,0] ; sumsq -> stat[:,1]
            nc.vector.tensor_reduce(stat[0:1, 0:1], s1ps, axis=mybir.AxisListType.X,
                                    op=OP.add)
            sqcol = const.tile([128, 1], f32, tag="sqcol")
            nc.vector.tensor_reduce(sqcol, sqacc, axis=mybir.AxisListType.X, op=OP.add)
            s2ps = pspool.tile([1, 1], f32, tag="s2ps")
            nc.tensor.matmul(s2ps, r(ones), r(sqcol), start=True, stop=True)
            nc.vector.tensor_copy(stat[0:1, 1:2], s2ps)

        # stats scalars (all on partition 0, then broadcast):
        # m = S1/N ; var = (S2 - N*m^2)/(N-1) ; inv_s = 1/sqrt(var)
        nc.vector.tensor_scalar_mul(stat[0:1, 2:3], stat[0:1, 0:1], 1.0 / N_ELEMS)  # mean
        nc.vector.tensor_mul(stat[0:1, 3:4], stat[0:1, 2:3], stat[0:1, 2:3])        # m^2
        nc.vector.tensor_scalar_mul(stat[0:1, 4:5], stat[0:1, 1:2], 1.0 / (N_ELEMS - 1.0))
        nc.vector.scalar_tensor_tensor(out=stat[0:1, 4:5], in0=stat[0:1, 3:4],
                                       scalar=-N_ELEMS / (N_ELEMS - 1.0),
                                       in1=stat[0:1, 4:5], op0=OP.mult, op1=OP.add)  # var
        # sqrt via ACT + one Newton step, then invert
        nc.scalar.activation(out=stat[0:1, 5:6], in_=stat[0:1, 4:5], func=AF.Sqrt)
        nc.vector.reciprocal(stat[0:1, 6:7], stat[0:1, 5:6])
        nc.vector.tensor_mul(stat[0:1, 6:7], stat[0:1, 6:7], stat[0:1, 4:5])  # var/s0
        nc.vector.tensor_add(stat[0:1, 5:6], stat[0:1, 5:6], stat[0:1, 6:7])
        nc.vector.tensor_scalar_mul(stat[0:1, 5:6], stat[0:1, 5:6], 0.5)      # s
        nc.vector.reciprocal(stat[0:1, 6:7], stat[0:1, 5:6])                  # 1/s
        # broadcast mean (col2) and inv_s (col6) to all partitions
        nc.gpsimd.partition_broadcast(stat[:, 2:3], stat[0:1, 2:3])
        nc.gpsimd.partition_broadcast(stat[:, 6:7], stat[0:1, 6:7])
        nc.vector.tensor_mul(stat[:, 7:8], stat[:, 2:3], stat[:, 6:7])
        nc.vector.tensor_scalar_mul(stat[:, 7:8], stat[:, 7:8], -1.0)         # -m/s

        # ---------------- phase A2: matmul1 + tanh -> h ----------------
        with ExitStack() as actx:
            xpool = actx.enter_context(tc.tile_pool(name="xb", bufs=6))
            wpool = actx.enter_context(tc.tile_pool(name="w1p", bufs=6))
            pspool = actx.enter_context(tc.tile_pool(name="psB", bufs=1, space="PSUM"))
            wsps = pspool.tile([128, MH], f32, tag="wsps")
            biasm = const.tile([128, MH], f32, tag="biasm")

            for half in range(BH):
                ups = [pspool.tile([128, 512], f32, tag=f"ups{m}", name=f"ups{m}") for m in range(MH)]
                for k in range(KC):
                    w1t = wpool.tile([128, HID], f32, tag="w1t")
                    nc.sync.dma_start(out=w1t, in_=w1[k * 128:(k + 1) * 128, :])
                    xt = xpool.tile([128, 512], f32, tag="xt")
                    nc.sync.dma_start(out=xt, in_=xT[k * 128:(k + 1) * 128,
                                                     half * 512:(half + 1) * 512])
                    for m in range(MH):
                        nc.tensor.matmul(ups[m], r(w1t[:, m * 128:(m + 1) * 128]),
                                         r(xt), start=(k == 0), stop=(k == KC - 1))
                    if half == 0:
                        # col-sums of w1 (for the mean-shift bias), once
                        for m in range(MH):
                            nc.tensor.matmul(wsps[:, m:m + 1],
                                             r(w1t[:, m * 128:(m + 1) * 128]), r(ones),
                                             start=(k == 0), stop=(k == KC - 1))
                if half == 0:
                    # bias_m = b1 + (-m/s)*w1sum  (per partition)
                    for m in range(MH):
                        nc.vector.scalar_tensor_tensor(
                            out=biasm[:, m:m + 1], in0=wsps[:, m:m + 1],
                            scalar=stat[:, 7:8], in1=b1sb[:, m:m + 1],
                            op0=OP.mult, op1=OP.add)
                for m in range(MH):
                    nc.scalar.activation(out=hres[m][:, half * 512:(half + 1) * 512],
                                         in_=ups[m], func=AF.Tanh,
                                         bias=biasm[:, m:m + 1], scale=stat[:, 6:7])

        # ---------------- phase B: matmul2 + spline ----------------
        BSP = 512                           # b-span per unit
        NB = B // BSP                       # 2
        bpool = ctx.enter_context(tc.tile_pool(name="w2p", bufs=8))
        pps = ctx.enter_context(tc.tile_pool(name="psC", bufs=3, space="PSUM"))
        ldps = ctx.enter_context(tc.tile_pool(name="psL", bufs=1, space="PSUM"))
        actp = ctx.enter_context(tc.tile_pool(name="actp", bufs=3))
        statep = ctx.enter_context(tc.tile_pool(name="statep", bufs=1))
        maskp = ctx.enter_context(tc.tile_pool(name="maskp", bufs=1))
        gathp = ctx.enter_context(tc.tile_pool(name="gathp", bufs=2))
        finp = ctx.enter_context(tc.tile_pool(name="finp", bufs=2))
        outp = ctx.enter_context(tc.tile_pool(name="outp", bufs=2))

        ldacc = [ldps.tile([1, BSP], f32, tag=f"ld{hh}", name=f"ld{hh}") for hh in range(NB)]
        n_ld = [0] * NB

        def mm2_plane(j, sp, half, out_act, bias_col):
            """matmul chunk (j, sp) for b-half -> ACT(tanh+bias) -> out_act."""
            ps = pps.tile([128, BSP], f32, tag="mm2ps")
            for hk in range(MH):
                w2t = bpool.tile([128, 128], f32, tag="w2t")
                nc.sync.dma_start(
                    out=w2t,
                    in_=w2p[hk * 128:(hk + 1) * 128,
                            j * S_LOC + sp * 128: j * S_LOC + (sp + 1) * 128])
                nc.tensor.matmul(ps, r(w2t), r(hres[hk][:, half * BSP:(half + 1) * BSP]),
                                 start=(hk == 0), stop=(hk == MH - 1))
            nc.scalar.activation(out=out_act, in_=ps, func=AF.Tanh,
                                 bias=bias_col, scale=1.0)

        for sp in range(SP_TILES):
            for half in range(NB):
                cw = statep.tile([128, K, BSP], f32, tag="cw")
                ch = statep.tile([128, K, BSP], f32, tag="ch")
                dpl = statep.tile([128, K, BSP], f32, tag="dpl")
                # --- w planes (net components K..2K) -> cumsum cw ---
                for jj in range(K):
                    j = K + jj
                    tw = actp.tile([128, BSP], f32, tag="tw")
                    mm2_plane(j, sp, half, tw, b2sb[:, j * SP_TILES + sp: j * SP_TILES + sp + 1])
                    te = actp.tile([128, BSP], f32, tag="te")
                    nc.scalar.activation(out=te, in_=tw, func=AF.Exp)
                    if jj == 0:
                        nc.vector.tensor_copy(cw[:, 0, :], te)
                    else:
                        nc.gpsimd.tensor_add(cw[:, jj, :], cw[:, jj - 1, :], te)
                # --- h planes (net components 0..K) -> cumsum ch ---
                for jj in range(K):
                    j = jj
                    tw = actp.tile([128, BSP], f32, tag="tw")
                    mm2_plane(j, sp, half, tw, b2sb[:, j * SP_TILES + sp: j * SP_TILES + sp + 1])
                    te = actp.tile([128, BSP], f32, tag="te")
                    nc.scalar.activation(out=te, in_=tw, func=AF.Exp)
                    if jj == 0:
                        nc.vector.tensor_copy(ch[:, 0, :], te)
                    else:
                        nc.gpsimd.tensor_add(ch[:, jj, :], ch[:, jj - 1, :], te)
                # --- d planes (net components 2K..3K) -> tanh only ---
                for jj in range(K):
                    j = 2 * K + jj
                    mm2_plane(j, sp, half, dpl[:, jj, :],
                              b2sb[:, j * SP_TILES + sp: j * SP_TILES + sp + 1])

                # --- binning ---
                xin = outp.tile([128, BSP], f32, tag="xin")
                nc.sync.dma_start(out=xin, in_=xinT[sp * 128:(sp + 1) * 128,
                                                    half * BSP:(half + 1) * BSP])
                xp = finp.tile([128, BSP], f32, tag="xp")
                nc.vector.scalar_tensor_tensor(out=xp, in0=xin, scalar=INV_2PI,
                                               in1=cw[:, K - 1, :], op0=OP.mult,
                                               op1=OP.mult)
                L = maskp.tile([128, K, BSP], dt.uint8, tag="L")
                for j in range(K):
                    nc.vector.tensor_tensor(out=L[:, j, :], in0=cw[:, j, :], in1=xp,
                                            op=OP.is_lt)

                # --- gathers via monotone predicated-copy chains ---
                cwk = gathp.tile([128, BSP], f32, tag="cwk")
                cwk1 = gathp.tile([128, BSP], f32, tag="cwk1")
                chk = gathp.tile([128, BSP], f32, tag="chk")
                chk1 = gathp.tile([128, BSP], f32, tag="chk1")
                dk = gathp.tile([128, BSP], f32, tag="dk")
                dk1 = gathp.tile([128, BSP], f32, tag="dk1")
                nc.vector.tensor_copy(cwk, cw[:, 0, :])
                nc.vector.memset(cwk1, 0.0)
                nc.vector.tensor_copy(chk, ch[:, 0, :])
                nc.vector.memset(chk1, 0.0)
                nc.vector.tensor_copy(dk, dpl[:, 0, :])
                nc.vector.tensor_copy(dk1, dpl[:, 1, :])
                for j in range(1, K):
                    mj = L[:, j - 1, :]
                    nc.vector.copy_predicated(cwk, mj, cw[:, j, :])
                    nc.vector.copy_predicated(cwk1, mj, cw[:, j - 1, :])
                    nc.vector.copy_predicated(chk, mj, ch[:, j, :])
                    nc.vector.copy_predicated(chk1, mj, ch[:, j - 1, :])
                    nc.vector.copy_predicated(dk, mj, dpl[:, j, :])
                    nc.vector.copy_predicated(dk1, mj, dpl[:, (j + 1) % K, :])

                # --- spline math (all planes [128, BSP]) ---
                # 8 reusable scratch slots t0..t7 + reuse of dead gather tiles
                t = [finp.tile([128, BSP], f32, tag=f"t{i}", name=f"t{i}")
                     for i in range(8)]
                Sw = cw[:, K - 1, :]
                Sh = ch[:, K - 1, :]
                nc.gpsimd.tensor_sub(t[1], cwk, cwk1)             # ewk
                nc.vector.reciprocal_approx_fast(out=t[2], in_=t[1])   # rw
                nc.gpsimd.tensor_sub(t[1], xp, cwk1)              # alpha num
                nc.vector.tensor_mul(t[1], t[1], t[2])            # alf = t1
                nc.gpsimd.tensor_mul(t[0], t[1], t[1])            # a2 = t0
                nc.vector.tensor_sub(t[3], t[1], t[0])            # a1m = t3
                nc.gpsimd.tensor_sub(t[4], chk, chk1)             # ehk = t4
                nc.vector.reciprocal_approx_fast(out=t[5], in_=Sh)     # rsh = t5
                nc.gpsimd.tensor_mul(t[6], Sw, t[5])
                nc.vector.tensor_mul(t[7], t[4], t[2])            # ehk*rw
                nc.vector.tensor_mul(t[6], t[6], t[7])            # sk = t6
                # softplus(x) = ln(1 + e^x) via Exp then Ln(in + 1)
                nc.scalar.activation(out=dk, in_=dk, func=AF.Exp)
                nc.scalar.activation(out=dk, in_=dk, func=AF.Ln, bias=1.0)
                nc.scalar.activation(out=dk1, in_=dk1, func=AF.Exp)
                nc.scalar.activation(out=dk1, in_=dk1, func=AF.Ln, bias=1.0)
                nc.gpsimd.tensor_add(t[7], dk, dk1)
                nc.vector.scalar_tensor_tensor(out=t[7], in0=t[6], scalar=-2.0,
                                               in1=t[7], op0=OP.mult, op1=OP.add)
                nc.gpsimd.tensor_mul(t[2], t[7], t[3])
                nc.vector.tensor_add(t[2], t[2], t[6])            # den = t2
                nc.vector.reciprocal_approx_fast(out=t[7], in_=t[2])   # rden = t7
                # phi:  A=cwk, Bt=cwk1 reused as scratch
                nc.vector.tensor_mul(cwk, t[6], t[0])             # sk*a2
                nc.gpsimd.tensor_mul(t[2], dk, t[3])              # dk*a1m
                nc.vector.tensor_add(cwk, cwk, t[2])
                nc.gpsimd.tensor_mul(cwk, cwk, t[7])
                nc.vector.tensor_mul(cwk, t[4], cwk)              # ehk*(...)
                nc.gpsimd.tensor_add(cwk, chk1, cwk)
                nc.vector.scalar_tensor_tensor(out=t[2], in0=cwk, scalar=TWO_PI,
                                               in1=t[5], op0=OP.mult, op1=OP.mult)
                nc.vector.tensor_scalar(out=t[2], in0=t[2], scalar1=phsb[:, 0:1],
                                        scalar2=None, op0=OP.add)  # phi1
                nc.gpsimd.tensor_scalar(out=cwk, in0=t[2], scalar1=TWO_PI,
                                        scalar2=None, op0=OP.is_ge)
                phiout = outp.tile([128, BSP], f32, tag="phiout")
                nc.vector.scalar_tensor_tensor(out=phiout, in0=cwk, scalar=-TWO_PI,
                                               in1=t[2], op0=OP.mult, op1=OP.add)
                nc.sync.dma_start(out=phiT[sp * 128:(sp + 1) * 128,
                                           half * BSP:(half + 1) * BSP], in_=phiout)
                # grad
                nc.vector.tensor_scalar(out=cwk, in0=t[1], scalar1=-1.0, scalar2=1.0,
                                        op0=OP.mult, op1=OP.add)   # 1-alpha
                nc.gpsimd.tensor_mul(cwk, cwk, cwk)                # (1-alpha)^2
                nc.vector.tensor_mul(t[1], dk1, t[0])              # dk1*a2
                nc.gpsimd.tensor_mul(chk, t[6], t[3])              # sk*a1m
                nc.vector.scalar_tensor_tensor(out=t[1], in0=chk, scalar=2.0,
                                               in1=t[1], op0=OP.mult, op1=OP.add)
                nc.gpsimd.tensor_mul(t[0], dk, cwk)
                nc.vector.tensor_add(t[0], t[0], t[1])             # bracket
                nc.gpsimd.tensor_mul(t[1], t[6], t[6])             # sk^2
                nc.vector.tensor_mul(t[1], t[1], t[0])
                nc.gpsimd.tensor_mul(t[0], t[7], t[7])             # rden^2
                nc.vector.tensor_mul(t[0], t[0], t[1])             # grad
                lg = outp.tile([128, BSP], f32, tag="lg")
                nc.scalar.activation(out=lg, in_=t[0], func=AF.Ln)
                nc.tensor.matmul(ldacc[half], r(ones), r(lg),
                                 start=(n_ld[half] == 0),
                                 stop=(n_ld[half] == SP_TILES - 1))
                n_ld[half] += 1

        for half in range(NB):
            ldout = outp.tile([1, BSP], f32, tag="ldout")
            nc.vector.tensor_copy(ldout, ldacc[half])
            nc.sync.dma_start(out=ldp[0:1, half * BSP:(half + 1) * BSP], in_=ldout)

    nc.compile()
    return nc


def _get_program():
    if "nc" not in _CACHE:
        _CACHE["nc"] = _build_program()
    return _CACHE["nc"]


def kernel(x_in, x_passive, log_density, w1, b1, w2, b2, phase_shift):
    from concourse.bass_utils import run_bass_kernel_spmd

    x_in = np.ascontiguousarray(x_in, np.float32)
    x_passive = np.ascontiguousarray(x_passive, np.float32)
    log_density = np.ascontiguousarray(log_density, np.float32)
    w1 = np.ascontiguousarray(w1, np.float32)
    b1 = np.ascontiguousarray(b1, np.float32)
    w2 = np.ascontiguousarray(w2, np.float32)
    b2 = np.ascontiguousarray(b2, np.float32)
    phase_shift = np.ascontiguousarray(phase_shift, np.float32)

    nc = _get_program()

    xT = np.ascontiguousarray(x_passive.T)                      # [S_IN, B]
    b1r = np.ascontiguousarray(b1.reshape(HID // 128, 128))
    ph = phase_shift.reshape(1, 1)
    w2r = w2.reshape(HID, S_OUT, NJ)
    b2r = b2.reshape(S_OUT, NJ)

    in_maps = []
    for c in range(NCORES):
        sl = slice(c * S_LOC, (c + 1) * S_LOC)
        w2c = np.ascontiguousarray(
            w2r[:, sl, :].transpose(0, 2, 1)).reshape(HID, NJ * S_LOC)
        b2c = np.ascontiguousarray(b2r[sl, :].T)                # [NJ, S_LOC]
        xinc = np.ascontiguousarray(x_in[:, sl].T)              # [S_LOC, B]
        in_maps.append({"xT": xT, "w1": w1, "b1r": b1r, "w2p": w2c,
                        "b2p": b2c, "xinT": xinc, "ph": ph})

    res = run_bass_kernel_spmd(nc, in_maps, core_ids=list(range(NCORES)))

    phi = np.empty((B, S_OUT), np.float32)
    ld_sum = np.zeros((B, 1), np.float32)
    for c in range(NCORES):
        sl = slice(c * S_LOC, (c + 1) * S_LOC)
        phi[:, sl] = res.results[c]["phiT"].T
        ld_sum += res.results[c]["ldp"].reshape(B, 1)
    ld = log_density - ld_sum
    return phi, ld


# revision 11
# speedup vs baseline: 1.3198x; 1.3198x over previous
"""Trainium2 Bass kernel for the CircularSplineLayer problem.

Strategy: model-parallel over S_OUT (each of the 8 cores owns a 512-column
slice of the spline outputs and the matching 1/8 slice of w2/b2), with the
first layer (global normalise + Linear+tanh over x_passive) computed
redundantly on every core so no cross-core collectives are needed.

Per-core device program:
  phase A: global mean/std of x_passive (PE ones-matmul sums + ACT Square
           accumulation), then h = tanh(((x-m)/s) @ w1 + b1) via the
           linearity trick h = tanh((x @ w1)/s + (b1 - (m/s)*colsum(w1))),
           keeping h resident in SBUF in [HID, B] (transposed) layout.
  phase B: second matmul out = h @ w2_slice with w2 columns permuted on the
           host so each 128-partition output chunk holds a single spline
           component plane [s x b], in processing order (w, h, d).  The
           rational-quadratic spline evaluation is then pure elementwise
           plane algebra: exp/cumsum over the 8 w/h components, monotone
           mask binning (cw_j < x), predicated-copy gathers, closed-form
           phi/grad.  log-density partials reduce over s (partition axis)
           via a PE ones-matmul.

All matmuls run in float32r (full-rate PE).  Host: shards/permutes inputs,
concatenates phi column slices, sums the 8 per-core log-grad partials.
"""

import numpy as np

B, S_IN, S_OUT, K, HID = 1024, 4096, 4096, 8, 512
NCORES = 8
S_LOC = S_OUT // NCORES          # 512 spline outputs per core
NJ = 3 * K                       # 24 component planes per s
SP_TILES = S_LOC // 128          # 4 partition tiles of s per core
TWO_PI = float(2.0 * np.pi)
INV_2PI = float(1.0 / TWO_PI)
N_ELEMS = float(B * S_IN)        # normalisation count for mean/std

# processing order of component planes: w (softmax widths), h, d
PLANE_ORDER = list(range(K, 2 * K)) + list(range(0, K)) + list(range(2 * K, 3 * K))

_CACHE = {}


def _build_program():
    import concourse.bass as bass
    import concourse.mybir as mybir
    from concourse import bacc
    from concourse import bass_isa
    from concourse.tile import TileContext
    from contextlib import ExitStack

    dt = mybir.dt
    f32 = dt.float32
    f32r = dt.float32r
    AF = mybir.ActivationFunctionType
    OP = mybir.AluOpType

    nc = bacc.Bacc("TRN2", target_bir_lowering=False, debug=False,
                   num_devices=NCORES)

    # matmul-facing inputs are declared float32r (same bits as float32)
    xT = nc.dram_tensor("xT", [S_IN, B], f32r, kind="ExternalInput").ap()
    w1 = nc.dram_tensor("w1", [S_IN, HID], f32r, kind="ExternalInput").ap()
    w2p = nc.dram_tensor("w2p", [HID, NJ * 128 * SP_TILES], f32r,
                         kind="ExternalInput").ap()
    b1r = nc.dram_tensor("b1r", [HID // 128, 128], f32, kind="ExternalInput").ap()
    b2p = nc.dram_tensor("b2p", [NJ, S_LOC], f32, kind="ExternalInput").ap()
    xinT = nc.dram_tensor("xinT", [S_LOC, B], f32, kind="ExternalInput").ap()
    ph = nc.dram_tensor("ph", [1, 1], f32, kind="ExternalInput").ap()
    onesd = nc.dram_tensor("onesd", [128, 1], f32r, kind="ExternalInput").ap()
    phiT = nc.dram_tensor("phiT", [S_LOC, B], f32, kind="ExternalOutput").ap()
    ldp = nc.dram_tensor("ldp", [1, B], f32, kind="ExternalOutput").ap()

    KC = S_IN // 128             # 32 contraction chunks for matmul 1
    MH = HID // 128              # 4 hid chunks
    BH = B // 512                # 2 b-halves
    JG = 4                       # component planes per w2 DMA group
    NG = NJ // JG                # 6 groups

    with TileContext(nc) as tc, ExitStack() as ctx:
        const = ctx.enter_context(tc.tile_pool(name="const", bufs=1))
        hpool = ctx.enter_context(tc.tile_pool(name="hres", bufs=1))

        ones = const.tile([128, 1], f32r, tag="ones")
        nc.sync.dma_start(out=ones, in_=onesd)
        b1sb = const.tile([128, MH], f32, tag="b1sb")
        for m in range(MH):
            nc.sync.dma_start(out=b1sb[:, m:m + 1],
                              in_=b1r[m, :].rearrange("(p one) -> p one", one=1))
        b2sb = const.tile([128, NJ * SP_TILES], f32, tag="b2sb")
        for j in range(NJ):
            for sp in range(SP_TILES):
                nc.sync.dma_start(
                    out=b2sb[:, j * SP_TILES + sp: j * SP_TILES + sp + 1],
                    in_=b2p[j, sp * 128:(sp + 1) * 128].rearrange("(p one) -> p one", one=1))
        phsb = const.tile([128, 1], f32, tag="phsb")
        nc.sync.dma_start(out=phsb, in_=ph.broadcast_to((128, 1)))

        # h kept resident for all of phase B: [HID,B] as 4 tiles of [128,B]
        hres = [hpool.tile([128, B], f32r, tag=f"h{m}", name=f"h{m}")
                for m in range(MH)]

        stat = const.tile([128, 8], f32, tag="stat")
        sqacc = const.tile([128, 2 * KC], f32, tag="sqacc")

        # ---------------- phase A: stats over x_passive ----------------
        with ExitStack() as actx:
            xpool = actx.enter_context(tc.tile_pool(name="xa", bufs=6))
            sqpool = actx.enter_context(tc.tile_pool(name="sq", bufs=3))
            pspool = actx.enter_context(tc.tile_pool(name="psA", bufs=1, space="PSUM"))

            s1ps = pspool.tile([1, 512], f32, tag="s1ps")
            i = 0
            for half in range(BH):
                for k in range(KC):
                    xt = xpool.tile([128, 512], f32r, tag="xt")
                    nc.sync.dma_start(out=xt, in_=xT[k * 128:(k + 1) * 128,
                                                     half * 512:(half + 1) * 512])
                    sq = sqpool.tile([128, 512], f32, tag="sq")
                    nc.scalar.activation(out=sq, in_=xt[:, :].bitcast(f32), func=AF.Square,
                                         accum_out=sqacc[:, i:i + 1])
                    nc.tensor.matmul(s1ps, ones, xt,
                                     start=(i == 0), stop=(i == 2 * KC - 1))
                    i += 1
            nc.vector.tensor_reduce(stat[0:1, 0:1], s1ps, axis=mybir.AxisListType.X,
                                    op=OP.add)
            sqcol = const.tile([128, 1], f32, tag="sqcol")
            nc.vector.tensor_reduce(sqcol, sqacc, axis=mybir.AxisListType.X, op=OP.add)
            # partition-axis reduce of the per-partition sumsq
            sqtot = const.tile([128, 1], f32, tag="sqtot")
            nc.gpsimd.partition_all_reduce(sqtot, sqcol, 128,
                                           bass_isa.ReduceOp.add)
            nc.vector.tensor_copy(stat[0:1, 1:2], sqtot[0:1, :])

        # stats scalars on partition 0, then broadcast:
        # m = S1/N ; var = (S2 - N*m^2)/(N-1) ; inv_s = 1/sqrt(var)
        nc.vector.tensor_scalar_mul(stat[0:1, 2:3], stat[0:1, 0:1], 1.0 / N_ELEMS)
        nc.vector.tensor_mul(stat[0:1, 3:4], stat[0:1, 2:3], stat[0:1, 2:3])
        nc.vector.tensor_scalar_mul(stat[0:1, 4:5], stat[0:1, 1:2], 1.0 / (N_ELEMS - 1.0))
        nc.vector.scalar_tensor_tensor(out=stat[0:1, 4:5], in0=stat[0:1, 3:4],
                                       scalar=-N_ELEMS / (N_ELEMS - 1.0),
                                       in1=stat[0:1, 4:5], op0=OP.mult, op1=OP.add)
        # s = sqrt(var) via ACT + one Newton step, then invert
        nc.scalar.activation(out=stat[0:1, 5:6], in_=stat[0:1, 4:5], func=AF.Sqrt)
        nc.vector.reciprocal(stat[0:1, 6:7], stat[0:1, 5:6])
        nc.vector.tensor_mul(stat[0:1, 6:7], stat[0:1, 6:7], stat[0:1, 4:5])
        nc.vector.tensor_add(stat[0:1, 5:6], stat[0:1, 5:6], stat[0:1, 6:7])
        nc.vector.tensor_scalar_mul(stat[0:1, 5:6], stat[0:1, 5:6], 0.5)
        nc.vector.reciprocal(stat[0:1, 6:7], stat[0:1, 5:6])
        nc.gpsimd.partition_broadcast(stat[:, 2:3], stat[0:1, 2:3])
        nc.gpsimd.partition_broadcast(stat[:, 6:7], stat[0:1, 6:7])
        nc.vector.tensor_mul(stat[:, 7:8], stat[:, 2:3], stat[:, 6:7])
        nc.vector.tensor_scalar_mul(stat[:, 7:8], stat[:, 7:8], -1.0)   # -m/s

        # ---------------- phase A2: matmul1 + tanh -> h ----------------
        with ExitStack() as actx:
            xpool = actx.enter_context(tc.tile_pool(name="xb", bufs=6))
            wpool = actx.enter_context(tc.tile_pool(name="w1p", bufs=6))
            pspool = actx.enter_context(tc.tile_pool(name="psB", bufs=1, space="PSUM"))
            wsps = pspool.tile([128, MH], f32, tag="wsps")
            biasm = const.tile([128, MH], f32, tag="biasm")

            for half in range(BH):
                ups = [pspool.tile([128, 512], f32, tag=f"ups{m}", name=f"ups{m}")
                       for m in range(MH)]
                for k in range(KC):
                    w1t = wpool.tile([128, HID], f32r, tag="w1t")
                    nc.sync.dma_start(out=w1t, in_=w1[k * 128:(k + 1) * 128, :])
                    xt = xpool.tile([128, 512], f32r, tag="xt")
                    nc.sync.dma_start(out=xt, in_=xT[k * 128:(k + 1) * 128,
                                                     half * 512:(half + 1) * 512])
                    for m in range(MH):
                        nc.tensor.matmul(ups[m], w1t[:, m * 128:(m + 1) * 128],
                                         xt, start=(k == 0), stop=(k == KC - 1))
                    if half == 0:
                        for m in range(MH):
                            nc.tensor.matmul(wsps[:, m:m + 1],
                                             w1t[:, m * 128:(m + 1) * 128].bitcast(f32),
                                             ones[:, :].bitcast(f32),
                                             start=(k == 0), stop=(k == KC - 1))
                if half == 0:
                    for m in range(MH):
                        nc.vector.scalar_tensor_tensor(
                            out=biasm[:, m:m + 1], in0=wsps[:, m:m + 1],
                            scalar=stat[:, 7:8], in1=b1sb[:, m:m + 1],
                            op0=OP.mult, op1=OP.add)
                for m in range(MH):
                    nc.scalar.activation(out=hres[m][:, half * 512:(half + 1) * 512],
                                         in_=ups[m], func=AF.Tanh,
                                         bias=biasm[:, m:m + 1], scale=stat[:, 6:7])

        # ---------------- phase B: matmul2 + spline ----------------
        BSP = 512
        NB = B // BSP
        bpool = ctx.enter_context(tc.tile_pool(name="w2s", bufs=2))
        pps = ctx.enter_context(tc.tile_pool(name="psC", bufs=3, space="PSUM"))
        ldps = ctx.enter_context(tc.tile_pool(name="psL", bufs=1, space="PSUM"))
        actp = ctx.enter_context(tc.tile_pool(name="actp", bufs=3))
        statep = ctx.enter_context(tc.tile_pool(name="statep", bufs=1))
        maskp = ctx.enter_context(tc.tile_pool(name="maskp", bufs=1))
        gathp = ctx.enter_context(tc.tile_pool(name="gathp", bufs=2))
        finp = ctx.enter_context(tc.tile_pool(name="finp", bufs=2))
        outp = ctx.enter_context(tc.tile_pool(name="outp", bufs=2))

        ldacc = [ldps.tile([1, BSP], f32, tag=f"ld{hh}", name=f"ld{hh}")
                 for hh in range(NB)]
        n_ld = [0] * NB

        for sp in range(SP_TILES):
            for half in range(NB):
                # stream this unit's w2 slice in NG groups of JG planes;
                # per group one DMA per hid chunk: [128, JG*128]
                w2g = {}
                for g in range(NG):
                    for hk in range(MH):
                        t = bpool.tile([128, JG * 128], f32r, tag=f"w2g{hk}",
                                       name=f"w2g{hk}_{g}")
                        col0 = sp * (NJ * 128) + g * (JG * 128)
                        nc.sync.dma_start(
                            out=t, in_=w2p[hk * 128:(hk + 1) * 128,
                                           col0: col0 + JG * 128])
                        w2g[(g, hk)] = t

                def mm2_plane(pos, out_act, bias_col):
                    """matmul for processing-position pos -> tanh+bias."""
                    ps = pps.tile([128, BSP], f32, tag="mm2ps", name="mm2ps")
                    g, r0 = divmod(pos, JG)
                    for hk in range(MH):
                        nc.tensor.matmul(
                            ps, w2g[(g, hk)][:, r0 * 128:(r0 + 1) * 128],
                            hres[hk][:, half * BSP:(half + 1) * BSP],
                            start=(hk == 0), stop=(hk == MH - 1))
                    nc.scalar.activation(out=out_act, in_=ps, func=AF.Tanh,
                                         bias=bias_col, scale=1.0)

                def bias_col(pos):
                    j = PLANE_ORDER[pos]
                    return b2sb[:, j * SP_TILES + sp: j * SP_TILES + sp + 1]

                cw = statep.tile([128, K, BSP], f32, tag="cw", name="cw")
                ch = statep.tile([128, K, BSP], f32, tag="ch", name="ch")
                dpl = statep.tile([128, K, BSP], f32, tag="dpl", name="dpl")
                # w planes (positions 0..7) -> cumsum cw
                for jj in range(K):
                    tw = actp.tile([128, BSP], f32, tag="tw", name="tw")
                    mm2_plane(jj, tw, bias_col(jj))
                    te = actp.tile([128, BSP], f32, tag="te", name="te")
                    nc.scalar.activation(out=te, in_=tw, func=AF.Exp)
                    if jj == 0:
                        nc.vector.tensor_copy(cw[:, 0, :], te)
                    else:
                        nc.gpsimd.tensor_add(cw[:, jj, :], cw[:, jj - 1, :], te)
                # h planes (positions 8..15) -> cumsum ch
                for jj in range(K):
                    tw = actp.tile([128, BSP], f32, tag="tw", name="tw")
                    mm2_plane(K + jj, tw, bias_col(K + jj))
                    te = actp.tile([128, BSP], f32, tag="te", name="te")
                    nc.scalar.activation(out=te, in_=tw, func=AF.Exp)
                    if jj == 0:
                        nc.vector.tensor_copy(ch[:, 0, :], te)
                    else:
                        nc.gpsimd.tensor_add(ch[:, jj, :], ch[:, jj - 1, :], te)
                # d planes (positions 16..23) -> tanh only
                for jj in range(K):
                    mm2_plane(2 * K + jj, dpl[:, jj, :], bias_col(2 * K + jj))

                # --- binning ---
                xin = outp.tile([128, BSP], f32, tag="xin", name="xin")
                nc.sync.dma_start(out=xin, in_=xinT[sp * 128:(sp + 1) * 128,
                                                    half * BSP:(half + 1) * BSP])
                xp = finp.tile([128, BSP], f32, tag="xp", name="xp")
                nc.vector.scalar_tensor_tensor(out=xp, in0=xin, scalar=INV_2PI,
                                               in1=cw[:, K - 1, :], op0=OP.mult,
                                               op1=OP.mult)
                L = maskp.tile([128, K, BSP], dt.uint8, tag="L", name="L")
                for j in range(K):
                    nc.vector.tensor_tensor(out=L[:, j, :], in0=cw[:, j, :], in1=xp,
                                            op=OP.is_lt)

                # --- gathers via monotone predicated-copy chains ---
                cwk = gathp.tile([128, BSP], f32, tag="cwk", name="cwk")
                cwk1 = gathp.tile([128, BSP], f32, tag="cwk1", name="cwk1")
                chk = gathp.tile([128, BSP], f32, tag="chk", name="chk")
                chk1 = gathp.tile([128, BSP], f32, tag="chk1", name="chk1")
                dk = gathp.tile([128, BSP], f32, tag="dk", name="dk")
                dk1 = gathp.tile([128, BSP], f32, tag="dk1", name="dk1")
                nc.vector.tensor_copy(cwk, cw[:, 0, :])
                nc.vector.memset(cwk1, 0.0)
                nc.vector.tensor_copy(chk, ch[:, 0, :])
                nc.vector.memset(chk1, 0.0)
                nc.vector.tensor_copy(dk, dpl[:, 0, :])
                nc.vector.tensor_copy(dk1, dpl[:, 1, :])
                for j in range(1, K):
                    mj = L[:, j - 1, :]
                    nc.vector.copy_predicated(cwk, mj, cw[:, j, :])
                    nc.vector.copy_predicated(cwk1, mj, cw[:, j - 1, :])
                    nc.vector.copy_predicated(chk, mj, ch[:, j, :])
                    nc.vector.copy_predicated(chk1, mj, ch[:, j - 1, :])
                    nc.vector.copy_predicated(dk, mj, dpl[:, j, :])
                    nc.vector.copy_predicated(dk1, mj, dpl[:, (j + 1) % K, :])

                # --- spline math (planes [128, BSP]) ---
                t = [finp.tile([128, BSP], f32, tag=f"t{i}", name=f"t{i}")
                     for i in range(8)]
                Sw = cw[:, K - 1, :]
                Sh = ch[:, K - 1, :]
                nc.gpsimd.tensor_sub(t[1], cwk, cwk1)             # ewk
                nc.vector.reciprocal_approx_fast(out=t[2], in_=t[1])   # rw
                nc.gpsimd.tensor_sub(t[1], xp, cwk1)              # alpha num
                nc.vector.tensor_mul(t[1], t[1], t[2])            # alf = t1
                nc.gpsimd.tensor_mul(t[0], t[1], t[1])            # a2 = t0
                nc.vector.tensor_sub(t[3], t[1], t[0])            # a1m = t3
                nc.gpsimd.tensor_sub(t[4], chk, chk1)             # ehk = t4
                nc.vector.reciprocal_approx_fast(out=t[5], in_=Sh)     # rsh = t5
                nc.gpsimd.tensor_mul(t[6], Sw, t[5])
                nc.vector.tensor_mul(t[7], t[4], t[2])            # ehk*rw
                nc.vector.tensor_mul(t[6], t[6], t[7])            # sk = t6
                # softplus(x) = ln(1 + e^x); both Exp ops precede the Lns so
                # the ACT table set switches once per unit, not four times
                nc.scalar.activation(out=dk, in_=dk, func=AF.Exp)
                nc.scalar.activation(out=dk1, in_=dk1, func=AF.Exp)
                nc.scalar.activation(out=dk, in_=dk, func=AF.Ln, bias=1.0)
                nc.scalar.activation(out=dk1, in_=dk1, func=AF.Ln, bias=1.0)
                nc.gpsimd.tensor_add(t[7], dk, dk1)
                nc.vector.scalar_tensor_tensor(out=t[7], in0=t[6], scalar=-2.0,
                                               in1=t[7], op0=OP.mult, op1=OP.add)
                nc.gpsimd.tensor_mul(t[2], t[7], t[3])
                nc.vector.tensor_add(t[2], t[2], t[6])            # den = t2
                nc.vector.reciprocal_approx_fast(out=t[7], in_=t[2])   # rden = t7
                # phi
                nc.vector.tensor_mul(cwk, t[6], t[0])             # sk*a2
                nc.gpsimd.tensor_mul(t[2], dk, t[3])              # dk*a1m
                nc.vector.tensor_add(cwk, cwk, t[2])
                nc.gpsimd.tensor_mul(cwk, cwk, t[7])
                nc.vector.tensor_mul(cwk, t[4], cwk)              # ehk*(...)
                nc.gpsimd.tensor_add(cwk, chk1, cwk)
                nc.vector.scalar_tensor_tensor(out=t[2], in0=cwk, scalar=TWO_PI,
                                               in1=t[5], op0=OP.mult, op1=OP.mult)
                nc.vector.tensor_scalar(out=t[2], in0=t[2], scalar1=phsb[:, 0:1],
                                        scalar2=None, op0=OP.add)  # phi1
                nc.vector.tensor_scalar(out=cwk, in0=t[2], scalar1=TWO_PI,
                                        scalar2=None, op0=OP.is_ge)
                phiout = outp.tile([128, BSP], f32, tag="phiout", name="phiout")
                nc.vector.scalar_tensor_tensor(out=phiout, in0=cwk, scalar=-TWO_PI,
                                               in1=t[2], op0=OP.mult, op1=OP.add)
                nc.sync.dma_start(out=phiT[sp * 128:(sp + 1) * 128,
                                           half * BSP:(half + 1) * BSP], in_=phiout)
                # grad
                nc.vector.tensor_scalar(out=cwk, in0=t[1], scalar1=-1.0, scalar2=1.0,
                                        op0=OP.mult, op1=OP.add)   # 1-alpha
                nc.gpsimd.tensor_mul(cwk, cwk, cwk)                # (1-alpha)^2
                nc.vector.tensor_mul(t[1], dk1, t[0])              # dk1*a2
                nc.gpsimd.tensor_mul(chk, t[6], t[3])              # sk*a1m
                nc.vector.scalar_tensor_tensor(out=t[1], in0=chk, scalar=2.0,
                                               in1=t[1], op0=OP.mult, op1=OP.add)
                nc.gpsimd.tensor_mul(t[0], dk, cwk)
                nc.vector.tensor_add(t[0], t[0], t[1])             # bracket
                nc.gpsimd.tensor_mul(t[1], t[6], t[6])             # sk^2
                nc.vector.tensor_mul(t[1], t[1], t[0])
                nc.gpsimd.tensor_mul(t[0], t[7], t[7])             # rden^2
                nc.vector.tensor_mul(t[0], t[0], t[1])             # grad
                lg = outp.tile([128, BSP], f32r, tag="lg", name="lg")
                nc.scalar.activation(out=lg, in_=t[0], func=AF.Ln)
                nc.tensor.matmul(ldacc[half], ones, lg,
                                 start=(n_ld[half] == 0),
                                 stop=(n_ld[half] == SP_TILES - 1))
                n_ld[half] += 1

        for half in range(NB):
            ldout = outp.tile([1, BSP], f32, tag="ldout", name="ldout")
            nc.vector.tensor_copy(ldout, ldacc[half])
            nc.sync.dma_start(out=ldp[0:1, half * BSP:(half + 1) * BSP], in_=ldout)

    nc.compile()
    return nc


def _get_program():
    if "nc" not in _CACHE:
        _CACHE["nc"] = _build_program()
    return _CACHE["nc"]


def _make_in_maps(x_in, x_passive, w1, b1, w2, b2, phase_shift):
    xT = np.ascontiguousarray(x_passive.T)                      # [S_IN, B]
    b1r = np.ascontiguousarray(b1.reshape(HID // 128, 128))
    ph = phase_shift.reshape(1, 1)
    w2r = w2.reshape(HID, S_OUT, NJ)
    b2r = b2.reshape(S_OUT, NJ)

    in_maps = []
    for c in range(NCORES):
        sl = slice(c * S_LOC, (c + 1) * S_LOC)
        # per-core w2 slice -> [HID, sp, plane-order pos, 128] column layout
        w2c = w2r[:, sl, :]                                     # [HID, S_LOC, NJ]
        w2c = w2c.reshape(HID, SP_TILES, 128, NJ)
        w2c = w2c[:, :, :, PLANE_ORDER]                         # processing order
        w2c = np.ascontiguousarray(w2c.transpose(0, 1, 3, 2)).reshape(
            HID, SP_TILES * NJ * 128)
        b2c = np.ascontiguousarray(b2r[sl, :].T)                # [NJ, S_LOC]
        xinc = np.ascontiguousarray(x_in[:, sl].T)              # [S_LOC, B]
        in_maps.append({"xT": xT, "w1": w1, "b1r": b1r, "w2p": w2c,
                        "b2p": b2c, "xinT": xinc, "ph": ph,
                        "onesd": np.ones((128, 1), np.float32)})
    return in_maps


def kernel(x_in, x_passive, log_density, w1, b1, w2, b2, phase_shift):
    from concourse.bass_utils import run_bass_kernel_spmd

    x_in = np.ascontiguousarray(x_in, np.float32)
    x_passive = np.ascontiguousarray(x_passive, np.float32)
    log_density = np.ascontiguousarray(log_density, np.float32)
    w1 = np.ascontiguousarray(w1, np.float32)
    b1 = np.ascontiguousarray(b1, np.float32)
    w2 = np.ascontiguousarray(w2, np.float32)
    b2 = np.ascontiguousarray(b2, np.float32)
    phase_shift = np.ascontiguousarray(phase_shift, np.float32)

    nc = _get_program()
    in_maps = _make_in_maps(x_in, x_passive, w1, b1, w2, b2, phase_shift)
    _CACHE["in_maps"] = in_maps

    res = run_bass_kernel_spmd(nc, in_maps, core_ids=list(range(NCORES)))

    phi = np.empty((B, S_OUT), np.float32)
    ld_sum = np.zeros((B, 1), np.float32)
    for c in range(NCORES):
        sl = slice(c * S_LOC, (c + 1) * S_LOC)
        phi[:, sl] = res.results[c]["phiT"].T
        ld_sum += res.results[c]["ldp"].reshape(B, 1)
    ld = log_density - ld_sum
    return phi, ld


# revision 13
# speedup vs baseline: 1.4356x; 1.0878x over previous
"""Trainium2 Bass kernel for the CircularSplineLayer problem.

Strategy: model-parallel over S_OUT (each of the 8 cores owns a 512-column
slice of the spline outputs and the matching 1/8 slice of w2/b2), with the
first layer (global normalise + Linear+tanh over x_passive) computed
redundantly on every core so no cross-core collectives are needed.

Per-core device program:
  phase A: global mean/std of x_passive (PE ones-matmul sums + ACT Square
           accumulation), then h = tanh(((x-m)/s) @ w1 + b1) via the
           linearity trick h = tanh((x @ w1)/s + (b1 - (m/s)*colsum(w1))),
           keeping h resident in SBUF in [HID, B] (transposed) layout.
  phase B: second matmul out = h @ w2_slice with w2 columns permuted on the
           host so each 128-partition output chunk holds a single spline
           component plane [s x b], in processing order (w, h, d).  The
           rational-quadratic spline evaluation is then pure elementwise
           plane algebra: exp/cumsum over the 8 w/h components, monotone
           mask binning (cw_j < x), predicated-copy gathers, closed-form
           phi/grad.  log-density partials reduce over s (partition axis)
           via a PE ones-matmul.

All matmuls run in float32r (full-rate PE).  Host: shards/permutes inputs,
concatenates phi column slices, sums the 8 per-core log-grad partials.
"""

import numpy as np

B, S_IN, S_OUT, K, HID = 1024, 4096, 4096, 8, 512
NCORES = 8
S_LOC = S_OUT // NCORES          # 512 spline outputs per core
NJ = 3 * K                       # 24 component planes per s
SP_TILES = S_LOC // 128          # 4 partition tiles of s per core
TWO_PI = float(2.0 * np.pi)
INV_2PI = float(1.0 / TWO_PI)
N_ELEMS = float(B * S_IN)        # normalisation count for mean/std

# processing order of component planes: w (softmax widths), h, d
PLANE_ORDER = list(range(K, 2 * K)) + list(range(0, K)) + list(range(2 * K, 3 * K))

_CACHE = {}


def _build_program():
    import concourse.bass as bass
    import concourse.mybir as mybir
    from concourse import bacc
    from concourse import bass_isa
    from concourse.tile import TileContext
    from contextlib import ExitStack

    dt = mybir.dt
    f32 = dt.float32
    f32r = dt.float32r
    AF = mybir.ActivationFunctionType
    OP = mybir.AluOpType

    nc = bacc.Bacc("TRN2", target_bir_lowering=False, debug=False,
                   num_devices=NCORES)

    # matmul-facing inputs are declared float32r (same bits as float32)
    xT = nc.dram_tensor("xT", [S_IN, B], f32r, kind="ExternalInput").ap()
    w1 = nc.dram_tensor("w1", [S_IN, HID], f32r, kind="ExternalInput").ap()
    w2p = nc.dram_tensor("w2p", [HID, NJ * 128 * SP_TILES], f32r,
                         kind="ExternalInput").ap()
    b1r = nc.dram_tensor("b1r", [HID // 128, 128], f32, kind="ExternalInput").ap()
    b2p = nc.dram_tensor("b2p", [NJ, S_LOC], f32, kind="ExternalInput").ap()
    xinT = nc.dram_tensor("xinT", [S_LOC, B], f32, kind="ExternalInput").ap()
    ph = nc.dram_tensor("ph", [1, 1], f32, kind="ExternalInput").ap()
    onesd = nc.dram_tensor("onesd", [128, 1], f32r, kind="ExternalInput").ap()
    phiT = nc.dram_tensor("phiT", [S_LOC, B], f32, kind="ExternalOutput").ap()
    ldp = nc.dram_tensor("ldp", [1, B], f32, kind="ExternalOutput").ap()

    KC = S_IN // 128             # 32 contraction chunks for matmul 1
    MH = HID // 128              # 4 hid chunks
    BH = B // 512                # 2 b-halves
    JG = 4                       # component planes per w2 DMA group
    NG = NJ // JG                # 6 groups

    with TileContext(nc) as tc, ExitStack() as ctx:
        const = ctx.enter_context(tc.tile_pool(name="const", bufs=1))
        hpool = ctx.enter_context(tc.tile_pool(name="hres", bufs=1))

        ones = const.tile([128, 1], f32r, tag="ones")
        nc.sync.dma_start(out=ones, in_=onesd)
        b1sb = const.tile([128, MH], f32, tag="b1sb")
        for m in range(MH):
            nc.sync.dma_start(out=b1sb[:, m:m + 1],
                              in_=b1r[m, :].rearrange("(p one) -> p one", one=1))
        b2sb = const.tile([128, NJ * SP_TILES], f32, tag="b2sb")
        for j in range(NJ):
            for sp in range(SP_TILES):
                nc.sync.dma_start(
                    out=b2sb[:, j * SP_TILES + sp: j * SP_TILES + sp + 1],
                    in_=b2p[j, sp * 128:(sp + 1) * 128].rearrange("(p one) -> p one", one=1))
        phsb = const.tile([128, 1], f32, tag="phsb")
        nc.sync.dma_start(out=phsb, in_=ph.broadcast_to((128, 1)))

        # h kept resident for all of phase B: [HID,B] as 4 tiles of [128,B]
        hres = [hpool.tile([128, B], f32r, tag=f"h{m}", name=f"h{m}")
                for m in range(MH)]

        stat = const.tile([128, 8], f32, tag="stat")
        sqacc = const.tile([128, 2 * KC], f32, tag="sqacc")

        # ---------------- phase A: stats over x_passive ----------------
        with ExitStack() as actx:
            xpool = actx.enter_context(tc.tile_pool(name="xa", bufs=6))
            sqpool = actx.enter_context(tc.tile_pool(name="sq", bufs=3))
            pspool = actx.enter_context(tc.tile_pool(name="psA", bufs=1, space="PSUM"))

            s1ps = pspool.tile([1, 512], f32, tag="s1ps")
            i = 0
            for half in range(BH):
                for k in range(KC):
                    xt = xpool.tile([128, 512], f32r, tag="xt")
                    nc.sync.dma_start(out=xt, in_=xT[k * 128:(k + 1) * 128,
                                                     half * 512:(half + 1) * 512])
                    sq = sqpool.tile([128, 512], f32, tag="sq")
                    nc.scalar.activation(out=sq, in_=xt[:, :].bitcast(f32), func=AF.Square,
                                         accum_out=sqacc[:, i:i + 1])
                    nc.tensor.matmul(s1ps, ones, xt,
                                     start=(i == 0), stop=(i == 2 * KC - 1))
                    i += 1
            nc.vector.tensor_reduce(stat[0:1, 0:1], s1ps, axis=mybir.AxisListType.X,
                                    op=OP.add)
            sqcol = const.tile([128, 1], f32, tag="sqcol")
            nc.vector.tensor_reduce(sqcol, sqacc, axis=mybir.AxisListType.X, op=OP.add)
            # partition-axis reduce of the per-partition sumsq
            sqtot = const.tile([128, 1], f32, tag="sqtot")
            nc.gpsimd.partition_all_reduce(sqtot, sqcol, 128,
                                           bass_isa.ReduceOp.add)
            nc.vector.tensor_copy(stat[0:1, 1:2], sqtot[0:1, :])

        # stats scalars on partition 0, then broadcast:
        # m = S1/N ; var = (S2 - N*m^2)/(N-1) ; inv_s = 1/sqrt(var)
        nc.vector.tensor_scalar_mul(stat[0:1, 2:3], stat[0:1, 0:1], 1.0 / N_ELEMS)
        nc.vector.tensor_mul(stat[0:1, 3:4], stat[0:1, 2:3], stat[0:1, 2:3])
        nc.vector.tensor_scalar_mul(stat[0:1, 4:5], stat[0:1, 1:2], 1.0 / (N_ELEMS - 1.0))
        nc.vector.scalar_tensor_tensor(out=stat[0:1, 4:5], in0=stat[0:1, 3:4],
                                       scalar=-N_ELEMS / (N_ELEMS - 1.0),
                                       in1=stat[0:1, 4:5], op0=OP.mult, op1=OP.add)
        # s = sqrt(var) via ACT + one Newton step, then invert
        nc.scalar.activation(out=stat[0:1, 5:6], in_=stat[0:1, 4:5], func=AF.Sqrt)
        nc.vector.reciprocal(stat[0:1, 6:7], stat[0:1, 5:6])
        nc.vector.tensor_mul(stat[0:1, 6:7], stat[0:1, 6:7], stat[0:1, 4:5])
        nc.vector.tensor_add(stat[0:1, 5:6], stat[0:1, 5:6], stat[0:1, 6:7])
        nc.vector.tensor_scalar_mul(stat[0:1, 5:6], stat[0:1, 5:6], 0.5)
        nc.vector.reciprocal(stat[0:1, 6:7], stat[0:1, 5:6])
        nc.gpsimd.partition_broadcast(stat[:, 2:3], stat[0:1, 2:3])
        nc.gpsimd.partition_broadcast(stat[:, 6:7], stat[0:1, 6:7])
        nc.vector.tensor_mul(stat[:, 7:8], stat[:, 2:3], stat[:, 6:7])
        nc.vector.tensor_scalar_mul(stat[:, 7:8], stat[:, 7:8], -1.0)   # -m/s

        # ---------------- phase A2: matmul1 + tanh -> h ----------------
        with ExitStack() as actx:
            xpool = actx.enter_context(tc.tile_pool(name="xb", bufs=6))
            wpool = actx.enter_context(tc.tile_pool(name="w1p", bufs=6))
            pspool = actx.enter_context(tc.tile_pool(name="psB", bufs=1, space="PSUM"))
            wsps = pspool.tile([128, MH], f32, tag="wsps")
            biasm = const.tile([128, MH], f32, tag="biasm")

            for half in range(BH):
                ups = [pspool.tile([128, 512], f32, tag=f"ups{m}", name=f"ups{m}")
                       for m in range(MH)]
                for k in range(KC):
                    w1t = wpool.tile([128, HID], f32r, tag="w1t")
                    nc.sync.dma_start(out=w1t, in_=w1[k * 128:(k + 1) * 128, :])
                    xt = xpool.tile([128, 512], f32r, tag="xt")
                    nc.sync.dma_start(out=xt, in_=xT[k * 128:(k + 1) * 128,
                                                     half * 512:(half + 1) * 512])
                    for m in range(MH):
                        nc.tensor.matmul(ups[m], w1t[:, m * 128:(m + 1) * 128],
                                         xt, start=(k == 0), stop=(k == KC - 1))
                    if half == 0:
                        for m in range(MH):
                            nc.tensor.matmul(wsps[:, m:m + 1],
                                             w1t[:, m * 128:(m + 1) * 128].bitcast(f32),
                                             ones[:, :].bitcast(f32),
                                             start=(k == 0), stop=(k == KC - 1))
                if half == 0:
                    for m in range(MH):
                        nc.vector.scalar_tensor_tensor(
                            out=biasm[:, m:m + 1], in0=wsps[:, m:m + 1],
                            scalar=stat[:, 7:8], in1=b1sb[:, m:m + 1],
                            op0=OP.mult, op1=OP.add)
                for m in range(MH):
                    nc.scalar.activation(out=hres[m][:, half * 512:(half + 1) * 512],
                                         in_=ups[m], func=AF.Tanh,
                                         bias=biasm[:, m:m + 1], scale=stat[:, 6:7])

        # ---------------- phase B: matmul2 + spline ----------------
        BSP = 512
        NB = B // BSP
        bpool = ctx.enter_context(tc.tile_pool(name="w2s", bufs=2))
        pps = ctx.enter_context(tc.tile_pool(name="psC", bufs=4, space="PSUM"))
        ldps = ctx.enter_context(tc.tile_pool(name="psL", bufs=1, space="PSUM"))
        actp = ctx.enter_context(tc.tile_pool(name="actp", bufs=3))
        statep = ctx.enter_context(tc.tile_pool(name="statep", bufs=2))
        dstatep = ctx.enter_context(tc.tile_pool(name="dstatep", bufs=1))
        maskp = ctx.enter_context(tc.tile_pool(name="maskp", bufs=1))
        lfp = ctx.enter_context(tc.tile_pool(name="lfp", bufs=2))
        gathp = ctx.enter_context(tc.tile_pool(name="gathp", bufs=2))
        finp = ctx.enter_context(tc.tile_pool(name="finp", bufs=1))
        outp = ctx.enter_context(tc.tile_pool(name="outp", bufs=2))

        ldacc = [ldps.tile([1, BSP], f32, tag=f"ld{hh}", name=f"ld{hh}")
                 for hh in range(NB)]
        n_ld = [0] * NB

        for sp in range(SP_TILES):
            for half in range(NB):
                # stream this unit's w2 slice in NG groups of JG planes;
                # per group one DMA per hid chunk: [128, JG*128]
                w2g = {}
                for g in range(NG):
                    for hk in range(MH):
                        t = bpool.tile([128, JG * 128], f32r, tag=f"w2g{hk}",
                                       name=f"w2g{hk}_{g}")
                        col0 = sp * (NJ * 128) + g * (JG * 128)
                        nc.sync.dma_start(
                            out=t, in_=w2p[hk * 128:(hk + 1) * 128,
                                           col0: col0 + JG * 128])
                        w2g[(g, hk)] = t

                def mm2_plane(pos, out_act, bias_col):
                    """matmul for processing-position pos -> tanh+bias."""
                    ps = pps.tile([128, BSP], f32, tag="mm2ps", name="mm2ps")
                    g, r0 = divmod(pos, JG)
                    for hk in range(MH):
                        nc.tensor.matmul(
                            ps, w2g[(g, hk)][:, r0 * 128:(r0 + 1) * 128],
                            hres[hk][:, half * BSP:(half + 1) * BSP],
                            start=(hk == 0), stop=(hk == MH - 1))
                    nc.scalar.activation(out=out_act, in_=ps, func=AF.Tanh,
                                         bias=bias_col, scale=1.0)

                def bias_col(pos):
                    j = PLANE_ORDER[pos]
                    return b2sb[:, j * SP_TILES + sp: j * SP_TILES + sp + 1]

                cwch = statep.tile([128, 2, K, BSP], f32, tag="cwch", name="cwch")
                cw = cwch[:, 0]
                ch = cwch[:, 1]
                dpl = dstatep.tile([128, K, BSP], dt.float16, tag="dpl", name="dpl")
                # w planes (positions 0..7) -> cumsum cw
                for jj in range(K):
                    tw = actp.tile([128, BSP], f32, tag="tw", name="tw")
                    mm2_plane(jj, tw, bias_col(jj))
                    te = actp.tile([128, BSP], f32, tag="te", name="te")
                    nc.scalar.activation(out=te, in_=tw, func=AF.Exp)
                    if jj == 0:
                        nc.vector.tensor_copy(cw[:, 0, :], te)
                    elif jj % 2 == 0:
                        nc.gpsimd.tensor_add(cw[:, jj, :], cw[:, jj - 1, :], te)
                    else:
                        nc.vector.tensor_add(cw[:, jj, :], cw[:, jj - 1, :], te)
                # h planes (positions 8..15) -> cumsum ch
                for jj in range(K):
                    tw = actp.tile([128, BSP], f32, tag="tw", name="tw")
                    mm2_plane(K + jj, tw, bias_col(K + jj))
                    te = actp.tile([128, BSP], f32, tag="te", name="te")
                    nc.scalar.activation(out=te, in_=tw, func=AF.Exp)
                    if jj == 0:
                        nc.vector.tensor_copy(ch[:, 0, :], te)
                    elif jj % 2 == 0:
                        nc.gpsimd.tensor_add(ch[:, jj, :], ch[:, jj - 1, :], te)
                    else:
                        nc.vector.tensor_add(ch[:, jj, :], ch[:, jj - 1, :], te)
                # d planes (positions 16..23) -> tanh only (fp16 storage)
                for jj in range(K):
                    mm2_plane(2 * K + jj, dpl[:, jj, :], bias_col(2 * K + jj))

                # --- binning ---
                xin = outp.tile([128, BSP], f32, tag="xin", name="xin")
                nc.sync.dma_start(out=xin, in_=xinT[sp * 128:(sp + 1) * 128,
                                                    half * BSP:(half + 1) * BSP])
                xp = finp.tile([128, BSP], f32, tag="xp", name="xp")
                nc.vector.scalar_tensor_tensor(out=xp, in0=xin, scalar=INV_2PI,
                                               in1=cw[:, K - 1, :], op0=OP.mult,
                                               op1=OP.mult)
                L = maskp.tile([128, K, BSP], dt.uint8, tag="L", name="L")
                for j in range(K):
                    nc.vector.tensor_tensor(out=L[:, j, :], in0=cw[:, j, :], in1=xp,
                                            op=OP.is_lt)

                # --- gathers: quad predicated-copy chains over (w/h, k-1/k) ---
                quad = gathp.tile([128, 2, 2, BSP], f32, tag="quad", name="quad")
                ddk = gathp.tile([128, 2, BSP], dt.float16, tag="ddk", name="ddk")
                nc.vector.memset(quad[:, :, 0, :], 0.0)
                nc.vector.tensor_copy(quad[:, :, 1, :], cwch[:, :, 0, :])
                nc.vector.tensor_copy(ddk, dpl[:, 0:2, :])
                for j in range(1, K):
                    mj = L[:, j - 1, :]
                    mj4 = bass.AP(tensor=mj.tensor, offset=mj.offset,
                                  ap=[mj.ap[0], [0, 2], [0, 2], mj.ap[1]])
                    nc.vector.copy_predicated(quad, mj4, cwch[:, :, j - 1:j + 1, :])
                    if j < K - 1:
                        mj2 = bass.AP(tensor=mj.tensor, offset=mj.offset,
                                      ap=[mj.ap[0], [0, 2], mj.ap[1]])
                        nc.vector.copy_predicated(ddk, mj2, dpl[:, j:j + 2, :])
                    else:
                        nc.vector.copy_predicated(ddk[:, 0, :], mj, dpl[:, K - 1, :])
                        nc.vector.copy_predicated(ddk[:, 1, :], mj, dpl[:, 0, :])
                cwk1 = quad[:, 0, 0, :]
                cwk = quad[:, 0, 1, :]
                chk1 = quad[:, 1, 0, :]
                chk = quad[:, 1, 1, :]

                # --- spline math (planes [128, BSP]) ---
                t = [finp.tile([128, BSP], f32, tag=f"t{i}", name=f"t{i}")
                     for i in range(8)]
                Sw = cw[:, K - 1, :]
                Sh = ch[:, K - 1, :]
                nc.gpsimd.tensor_sub(t[1], cwk, cwk1)             # ewk
                nc.vector.reciprocal_approx_fast(out=t[2], in_=t[1])   # rw
                nc.gpsimd.tensor_sub(t[1], xp, cwk1)              # alpha num
                nc.vector.tensor_mul(t[1], t[1], t[2])            # alf = t1
                nc.scalar.activation(out=t[0], in_=t[1], func=AF.Square)  # a2
                nc.vector.tensor_sub(t[3], t[1], t[0])            # a1m = t3
                nc.gpsimd.tensor_sub(t[4], chk, chk1)             # ehk = t4
                nc.vector.reciprocal_approx_fast(out=t[5], in_=Sh)     # rsh = t5
                nc.gpsimd.tensor_mul(t[6], Sw, t[5])
                nc.vector.tensor_mul(t[7], t[4], t[2])            # ehk*rw
                nc.vector.tensor_mul(t[6], t[6], t[7])            # sk = t6
                # softplus(d) = ln(1 + e^d); Exps first, Lns together
                dkf = cwk     # w-side quad planes are dead: reuse as f32 scratch
                dk1f = chk
                nc.scalar.activation(out=dkf, in_=ddk[:, 0, :], func=AF.Exp)
                nc.scalar.activation(out=dk1f, in_=ddk[:, 1, :], func=AF.Exp)
                nc.scalar.activation(out=dkf, in_=dkf, func=AF.Ln, bias=1.0)
                nc.scalar.activation(out=dk1f, in_=dk1f, func=AF.Ln, bias=1.0)
                nc.gpsimd.tensor_add(t[7], dkf, dk1f)
                nc.vector.scalar_tensor_tensor(out=t[7], in0=t[6], scalar=-2.0,
                                               in1=t[7], op0=OP.mult, op1=OP.add)
                nc.gpsimd.tensor_mul(t[2], t[7], t[3])
                nc.vector.tensor_add(t[2], t[2], t[6])            # den = t2
                nc.vector.reciprocal_approx_fast(out=t[7], in_=t[2])   # rden = t7
                # phi
                nc.vector.tensor_mul(t[2], t[6], t[0])            # sk*a2
                nc.gpsimd.tensor_mul(xp, dkf, t[3])               # dk*a1m (xp dead)
                nc.vector.tensor_add(t[2], t[2], xp)
                nc.gpsimd.tensor_mul(t[2], t[2], t[7])
                nc.vector.tensor_mul(t[2], t[4], t[2])            # ehk*(...)
                nc.gpsimd.tensor_add(t[2], chk1, t[2])            # chk1 dead after
                nc.vector.scalar_tensor_tensor(out=t[4], in0=t[2], scalar=TWO_PI,
                                               in1=t[5], op0=OP.mult, op1=OP.mult)
                nc.vector.tensor_scalar(out=t[4], in0=t[4], scalar1=phsb[:, 0:1],
                                        scalar2=None, op0=OP.add)  # phi1
                nc.vector.tensor_scalar(out=t[2], in0=t[4], scalar1=TWO_PI,
                                        scalar2=None, op0=OP.is_ge)
                phiout = outp.tile([128, BSP], f32, tag="phiout", name="phiout")
                nc.vector.scalar_tensor_tensor(out=phiout, in0=t[2], scalar=-TWO_PI,
                                               in1=t[4], op0=OP.mult, op1=OP.add)
                nc.sync.dma_start(out=phiT[sp * 128:(sp + 1) * 128,
                                           half * BSP:(half + 1) * BSP], in_=phiout)
                # grad
                nc.scalar.activation(out=t[2], in_=t[1], func=AF.Square,
                                     scale=-1.0, bias=1.0)         # (1-alpha)^2
                nc.vector.tensor_mul(t[1], dk1f, t[0])             # dk1*a2
                nc.gpsimd.tensor_mul(t[0], t[6], t[3])             # sk*a1m
                nc.vector.scalar_tensor_tensor(out=t[1], in0=t[0], scalar=2.0,
                                               in1=t[1], op0=OP.mult, op1=OP.add)
                nc.gpsimd.tensor_mul(t[0], dkf, t[2])              # dk*(1-a)^2
                nc.vector.tensor_add(t[0], t[0], t[1])             # bracket
                nc.scalar.activation(out=t[1], in_=t[6], func=AF.Square)  # sk^2
                nc.gpsimd.tensor_mul(t[1], t[1], t[0])
                nc.scalar.activation(out=t[0], in_=t[7], func=AF.Square)  # rden^2
                nc.vector.tensor_mul(t[0], t[0], t[1])             # grad
                lg = outp.tile([128, BSP], f32r, tag="lg", name="lg")
                nc.scalar.activation(out=lg, in_=t[0], func=AF.Ln)
                nc.tensor.matmul(ldacc[half], ones, lg,
                                 start=(n_ld[half] == 0),
                                 stop=(n_ld[half] == SP_TILES - 1))
                n_ld[half] += 1

        for half in range(NB):
            ldout = outp.tile([1, BSP], f32, tag="ldout", name="ldout")
            nc.vector.tensor_copy(ldout, ldacc[half])
            nc.sync.dma_start(out=ldp[0:1, half * BSP:(half + 1) * BSP], in_=ldout)

    nc.compile()
    return nc


def _get_program():
    if "nc" not in _CACHE:
        _CACHE["nc"] = _build_program()
    return _CACHE["nc"]


def _make_in_maps(x_in, x_passive, w1, b1, w2, b2, phase_shift):
    xT = np.ascontiguousarray(x_passive.T)                      # [S_IN, B]
    b1r = np.ascontiguousarray(b1.reshape(HID // 128, 128))
    ph = phase_shift.reshape(1, 1)
    w2r = w2.reshape(HID, S_OUT, NJ)
    b2r = b2.reshape(S_OUT, NJ)

    in_maps = []
    for c in range(NCORES):
        sl = slice(c * S_LOC, (c + 1) * S_LOC)
        # per-core w2 slice -> [HID, sp, plane-order pos, 128] column layout
        w2c = w2r[:, sl, :]                                     # [HID, S_LOC, NJ]
        w2c = w2c.reshape(HID, SP_TILES, 128, NJ)
        w2c = w2c[:, :, :, PLANE_ORDER]                         # processing order
        w2c = np.ascontiguousarray(w2c.transpose(0, 1, 3, 2)).reshape(
            HID, SP_TILES * NJ * 128)
        b2c = np.ascontiguousarray(b2r[sl, :].T)                # [NJ, S_LOC]
        xinc = np.ascontiguousarray(x_in[:, sl].T)              # [S_LOC, B]
        in_maps.append({"xT": xT, "w1": w1, "b1r": b1r, "w2p": w2c,
                        "b2p": b2c, "xinT": xinc, "ph": ph,
                        "onesd": np.ones((128, 1), np.float32)})
    return in_maps


def kernel(x_in, x_passive, log_density, w1, b1, w2, b2, phase_shift):
    from concourse.bass_utils import run_bass_kernel_spmd

    x_in = np.ascontiguousarray(x_in, np.float32)
    x_passive = np.ascontiguousarray(x_passive, np.float32)
    log_density = np.ascontiguousarray(log_density, np.float32)
    w1 = np.ascontiguousarray(w1, np.float32)
    b1 = np.ascontiguousarray(b1, np.float32)
    w2 = np.ascontiguousarray(w2, np.float32)
    b2 = np.ascontiguousarray(b2, np.float32)
    phase_shift = np.ascontiguousarray(phase_shift, np.float32)

    nc = _get_program()
    in_maps = _make_in_maps(x_in, x_passive, w1, b1, w2, b2, phase_shift)
    _CACHE["in_maps"] = in_maps

    res = run_bass_kernel_spmd(nc, in_maps, core_ids=list(range(NCORES)))

    phi = np.empty((B, S_OUT), np.float32)
    ld_sum = np.zeros((B, 1), np.float32)
    for c in range(NCORES):
        sl = slice(c * S_LOC, (c + 1) * S_LOC)
        phi[:, sl] = res.results[c]["phiT"].T
        ld_sum += res.results[c]["ldp"].reshape(B, 1)
    ld = log_density - ld_sum
    return phi, ld
